# revision 21
# baseline (speedup 1.0000x reference)
"""GroupedQueryAttention (B=2, N=2048, D=2048, H=16, HKV=4, HD=128) on 8 trn2 cores.

Sharding: core c handles (batch b = c//4, kv-head g = c%4): 4 q-heads + 1 kv head.
RoPE (with the reference's sin==cos quirk) is folded into Wq/Wk host-side.
The softmax scale is folded into Wq. All matmuls run in bf16 with fp32 PSUM
accumulation, feed 512 (PSUM bank limit), kd-outer loops for stationary reuse.

Attention is restructured around engine balance (PE 2.4GHz, Scalar 1.2GHz):
per (head, n-half of 1024): scores for an ncx-pair land in one 2-bank PSUM
tile [128,1024]; ONE wide exp on ScalarE covers both (amortizes per-op
overhead); softmax denominators come from DVE bf16 accumulation of the exp
tiles (replacing 256 all-ones PE matmuls with 16) + one ones-matmul per
n-chunk; PV matmuls are software-pipelined one mt behind scores so the PE
never waits on the same-iteration exp. Per-head chunked AllGathers (bf16)
overlap with later heads' attention; gathered slabs prefetch into 4 SBUF
buffers; the output projection accumulates into a transposed [d, n] fp32
SBUF accumulator (aliased over x's SBUF slot), and the host transposes each
core's [512, 2048] slice back.
Host gathers: out[b][:, g*512:(g+1)*512] = core (b,g) output transposed.
"""

import sys
import types

import numpy as np

B, N, D = 2, 2048, 2048
H, HKV, HD = 16, 4, 128
G = H // HKV  # q heads per kv head = 4
N_CORES = 8
ROPE_BASE = 10000.0
DSLICE = D // G  # 512 output columns per core
JL = G * HD  # 512 local attention-output rows per core


def _install_axon_ntff_hook():
    """This container's antenv lacks axon_hooks; inject it so trace=True works."""
    if "antenv.axon_hooks" in sys.modules:
        return
    try:
        from trn_agent_boot.trn_boot import _ntff_profile_via_ctypes

        hook = _ntff_profile_via_ctypes("/opt/axon/libaxon_pjrt.so")
    except Exception:
        hook = None
    mod = types.ModuleType("antenv.axon_hooks")
    mod.get_axon_ntff_profile_hook = lambda: hook
    mod.set_axon_ntff_profile_hook = lambda h: None
    sys.modules["antenv.axon_hooks"] = mod


def _fold_rope(w: np.ndarray, n_heads: int) -> np.ndarray:
    """Return W' with the (sin==cos) RoPE mixing folded in: x@W' = M(x@W) per head."""
    wf = w.reshape(D, n_heads, HD)
    lo, hi = wf[..., : HD // 2], wf[..., HD // 2 :]
    return np.concatenate([lo - hi, hi + lo], axis=-1).reshape(D, n_heads * HD)


def _cos_table() -> np.ndarray:
    inv_freq = 1.0 / (ROPE_BASE ** (np.arange(0, HD, 2, dtype=np.float64) / HD))
    freqs = np.arange(N, dtype=np.float64)[:, None] * inv_freq[None, :]  # [N, 64]
    emb = np.concatenate([freqs, freqs], axis=-1)  # [N, 128]
    return np.cos(emb).T.astype(np.float32).copy()  # [128, N]


_NC_CACHE: dict = {}


def _build_nc():
    if "nc" in _NC_CACHE:
        return _NC_CACHE["nc"]

    import concourse.bacc as bacc
    import concourse.mybir as mybir
    import concourse.tile as tile
    from concourse.bass import ts
    from concourse.masks import make_identity

    f32 = mybir.dt.float32
    bf16 = mybir.dt.bfloat16
    AFT = mybir.ActivationFunctionType
    KD = D // 128  # 16 contraction chunks
    NT = N // 128  # 16 m tiles of 128
    NC512 = N // 512  # 4 chunks of 512
    DC = DSLICE // 128  # 4 output-column chunks of 128

    nc = bacc.Bacc(target_bir_lowering=False, debug=False, num_devices=N_CORES)

    xt = nc.dram_tensor("xt", [D, N], bf16, kind="ExternalInput")  # x[b].T
    wq = nc.dram_tensor("wq", [D, JL], bf16, kind="ExternalInput")  # folded+scaled
    wk = nc.dram_tensor("wk", [D, HD], bf16, kind="ExternalInput")  # folded
    wv = nc.dram_tensor("wv", [D, HD], bf16, kind="ExternalInput")
    wo = nc.dram_tensor("wo", [H * HD, DSLICE], bf16, kind="ExternalInput")
    cost = nc.dram_tensor("cost", [HD, N], f32, kind="ExternalInput")
    # transposed output: outT[d, n]; host transposes back
    out = nc.dram_tensor("out", [DSLICE, N], f32, kind="ExternalOutput")

    xt_v = xt.rearrange("(ko p) n -> p ko n", p=128)
    wq_v = wq.rearrange("(ko p) j -> p ko j", p=128)
    wk_v = wk.rearrange("(ko p) j -> p ko j", p=128)
    wv_v = wv.rearrange("(ko p) j -> p ko j", p=128)
    wo_v = wo.rearrange("(ko p) d -> p ko d", p=128)

    with tile.TileContext(nc) as tc:
        with (
            tc.tile_pool(name="big", bufs=1) as big_pool,
            tc.tile_pool(name="ag", bufs=3) as ag_pool,
            tc.tile_pool(name="otn", bufs=4) as otn_pool,
            tc.tile_pool(name="wpool", bufs=1) as w_pool,
            tc.tile_pool(name="work", bufs=1) as work_pool,
            tc.tile_pool(name="st", bufs=4) as st_pool,
            tc.tile_pool(name="accp", bufs=2) as acc_pool,
            tc.tile_pool(name="ev", bufs=2) as ev_pool,
            tc.tile_pool(name="spair", bufs=2, space="PSUM") as sp_pool,
            tc.tile_pool(name="otp", bufs=2, space="PSUM") as ot_pool,
            tc.tile_pool(name="misc", bufs=2, space="PSUM") as misc_pool,
            tc.tile_pool(name="dram", bufs=1, space="DRAM") as dram_pool,
        ):
            # ---- persistent SBUF tensors ----
            x_sb = big_pool.tile([128, KD, N], bf16, tag="big")
            wq_sb = w_pool.tile([128, KD, JL], bf16, tag="wq")
            wk_sb = w_pool.tile([128, KD, HD], bf16, tag="wk")
            wv_sb = w_pool.tile([128, KD, HD], bf16, tag="wv")
            wo_sb = w_pool.tile([128, KD, DSLICE], bf16, tag="wo")
            cos_sb = w_pool.tile([128, N], f32, tag="cos")
            qT_sb = work_pool.tile([128, G, N], bf16, tag="qT")
            kT_sb = work_pool.tile([128, N], bf16, tag="kT")
            vT_sb = ag_pool.tile([128, HKV, N], bf16, tag="agsb", name="vT_sb")[:, 0, :]
            v_sb = work_pool.tile([128, N], bf16, tag="v")  # [m-part, mt*128+hd]
            ones_sb = work_pool.tile([128, 128], bf16, tag="ones")
            ident_sb = work_pool.tile([128, 128], bf16, tag="ident")

            nc.gpsimd.memset(ones_sb[:], 1.0)
            make_identity(nc, ident_sb[:])

            # ---- input DMAs (weights needed first come first) ----
            nc.sync.dma_start(wk_sb[:], wk_v[:])
            for kd in range(4):
                nc.sync.dma_start(x_sb[:, kd, :], xt_v[:, kd, :])
            nc.sync.dma_start(cos_sb[:], cost[:, :])
            nc.sync.dma_start(wv_sb[:], wv_v[:])
            for kd in range(4, KD):
                nc.sync.dma_start(x_sb[:, kd, :], xt_v[:, kd, :])
            nc.sync.dma_start(wq_sb[:], wq_v[:])
            nc.sync.dma_start(wo_sb[:], wo_v[:])

            def proj_pair(lhs_chunks, dst_wide_op, name):
                """kd-outer projection into 2 spair tiles (4 bank-groups);
                dst_wide_op(j, psum_tile_1024) consumes each half."""
                psA = sp_pool.tile([128, 1024], f32, tag="spair", name=f"{name}A")
                psB = sp_pool.tile([128, 1024], f32, tag="spair", name=f"{name}B")
                quads = [(psA, 0), (psA, 512), (psB, 0), (psB, 512)]
                for kd in range(KD):
                    for j, (pst, off) in enumerate(quads):
                        nc.tensor.matmul(
                            pst[:, off : off + 512],
                            lhsT=lhs_chunks(kd),
                            rhs=x_sb[:, kd, ts(j, 512)],
                            start=(kd == 0),
                            stop=(kd == KD - 1),
                        )
                dst_wide_op(0, psA)
                dst_wide_op(1, psB)

            # k: kT = cos * (Wk.T @ x)
            proj_pair(
                lambda kd: wk_sb[:, kd, :],
                lambda j, ps: nc.vector.tensor_mul(
                    kT_sb[:, ts(j, 1024)], ps, cos_sb[:, ts(j, 1024)]
                ),
                "kproj",
            )

            # v: vT then PE-transpose into natural [m, hd] layout
            proj_pair(
                lambda kd: wv_sb[:, kd, :],
                lambda j, ps: nc.vector.tensor_copy(vT_sb[:, ts(j, 1024)], ps),
                "vproj",
            )
            for q4 in range(NT // 4):
                trf = misc_pool.tile([128, 512], f32, tag="misc", name="tr")
                trb = trf[:, 0:256].bitcast(bf16)
                for j in range(4):
                    mt = q4 * 4 + j
                    nc.tensor.transpose(
                        trb[:, ts(j, 128)], vT_sb[:, ts(mt, 128)], ident_sb[:]
                    )
                nc.vector.tensor_copy(v_sb[:, ts(q4, 512)], trb[:])

            # q heads 0 and 3 sequential; heads 1 and 2 are interleaved into
            # attention as filler matmuls (see qproj_fillers)
            for h in (0, 3):
                proj_pair(
                    lambda kd, h=h: wq_sb[:, kd, ts(h, 128)],
                    lambda j, ps, h=h: nc.vector.tensor_mul(
                        qT_sb[:, h, ts(j, 1024)], ps, cos_sb[:, ts(j, 1024)]
                    ),
                    f"qproj{h}",
                )

            # outT fp32 accumulator for the output projection: aliases x_sb's
            # SBUF slot (same tag, bufs=1); x is dead after qproj.
            outT_acc = big_pool.tile([128, DC, N], f32, tag="big")

            # ---- collective staging ----
            # staging chunks: head h is gathered in CHUNKS[h] pieces so ring
            # latency overlaps production (smaller chunks for later heads)
            CHUNKS = [1, 2, 2, 4]
            ag_ins = []   # [h][c] -> dram [HD, chunk_n]
            ag_outs = []  # [h][c] -> dram [HKV*HD, chunk_n]
            for h in range(G):
                cn = N // CHUNKS[h]
                ag_ins.append(
                    [
                        dram_pool.tile(
                            [HD, cn], bf16, tag=f"agi{h}_{c}", name=f"agi{h}_{c}"
                        )
                        for c in range(CHUNKS[h])
                    ]
                )
                ag_outs.append(
                    [
                        dram_pool.tile(
                            [HKV * HD, cn],
                            bf16,
                            tag=f"ago{h}_{c}",
                            name=f"ago{h}_{c}",
                        )
                        for c in range(CHUNKS[h])
                    ]
                )

            # leading tiny collective: absorbs cross-core rendezvous skew while
            # the PE is busy with projections, so the first real gather is cheap
            bar_in = dram_pool.tile([1, 128], bf16, tag="bar_in", name="bar_in")
            bar_out = dram_pool.tile([4, 128], bf16, tag="bar_out", name="bar_out")
            nc.gpsimd.collective_compute(
                "AllGather",
                mybir.AluOpType.bypass,
                replica_groups=[[0, 1, 2, 3], [4, 5, 6, 7]],
                ins=[bar_in[:].opt()],
                outs=[bar_out[:].opt()],
            )

            ag_tiles = {}

            def qproj_fillers(h):
                """32 closures: qproj for head h via misc-pool kd-groups.
                4 ncx-groups of 16 matmuls, 2 per closure (8 closures/group);
                groups close before half boundaries so misc rotation is safe."""
                fns = []
                for ncx in range(NC512):
                    qps = misc_pool.tile(
                        [128, 512], f32, tag="misc", name=f"qf{h}{ncx}"
                    )
                    for k2 in range(8):

                        def emit(h=h, ncx=ncx, k2=k2, qps=qps):
                            for kd in (2 * k2, 2 * k2 + 1):
                                nc.tensor.matmul(
                                    qps,
                                    lhsT=wq_sb[:, kd, ts(h, 128)],
                                    rhs=x_sb[:, kd, ts(ncx, 512)],
                                    start=(kd == 0),
                                    stop=(kd == KD - 1),
                                )
                            if k2 == 7:
                                nc.vector.tensor_mul(
                                    qT_sb[:, h, ts(ncx, 512)],
                                    qps,
                                    cos_sb[:, ts(ncx, 512)],
                                )

                        fns.append(emit)
                return fns

            def slab_fillers(h, nn_range=None):
                """Closures: output-projection slab for head h; each closure is
                one complete 4-matmul misc-pool group + DVE accumulate. For
                h==3 the gathered slab lives in two n-chunk tiles. Loops nn
                outer so chunk-0 groups come first (gather-chunk gating)."""
                fns = []
                for nn in nn_range if nn_range is not None else range(NC512):
                    for dc in range(DC):

                        def emit(h=h, dc=dc, nn=nn):
                            ps = misc_pool.tile(
                                [128, 512], f32, tag="misc", name=f"sd{h}"
                            )
                            for r in range(HKV):
                                jc = r * G + h
                                rhs = ag_tiles[h][:, r, ts(nn, 512)]
                                nc.tensor.matmul(
                                    ps,
                                    lhsT=wo_sb[:, jc, ts(dc, 128)],
                                    rhs=rhs,
                                    start=(r == 0),
                                    stop=(r == HKV - 1),
                                )
                            if h == 0:
                                nc.vector.tensor_copy(outT_acc[:, dc, ts(nn, 512)], ps)
                            else:
                                nc.vector.tensor_add(
                                    outT_acc[:, dc, ts(nn, 512)],
                                    ps,
                                    outT_acc[:, dc, ts(nn, 512)],
                                )
                            if h == G - 1 and nn == 1:
                                nc.sync.dma_start(
                                    out[ts(dc, 128), 0:1024],
                                    outT_acc[:, dc, 0:1024],
                                )
                            if h == G - 1 and nn == NC512 - 1:
                                nc.sync.dma_start(
                                    out[ts(dc, 128), 1024:2048],
                                    outT_acc[:, dc, 1024:2048],
                                )

                        fns.append(emit)
                return fns

            def attn_head(h, fillers, filler_iters):
                """Attention for head h; drains one filler closure per
                iteration index listed in filler_iters (0..31)."""
                it = 0
                fq = list(fillers)
                for half in range(2):
                    ot_a = ot_pool.tile([128, 512], f32, tag="otp", name=f"ota{h}{half}")
                    ot_b = ot_pool.tile([128, 512], f32, tag="otp", name=f"otb{h}{half}")
                    acc = acc_pool.tile([128, 1024], bf16, tag="acc", name=f"acc{h}{half}")
                    sts = [None] * NT
                    for mt in range(NT):
                        sp = sp_pool.tile(
                            [128, 1024], f32, tag="spair", name=f"sp{h}{half}"
                        )
                        nc.tensor.matmul(
                            sp[:, 0:512],
                            lhsT=kT_sb[:, ts(mt, 128)],
                            rhs=qT_sb[:, h, ts(2 * half, 512)],
                            start=True,
                            stop=True,
                        )
                        nc.tensor.matmul(
                            sp[:, 512:1024],
                            lhsT=kT_sb[:, ts(mt, 128)],
                            rhs=qT_sb[:, h, ts(2 * half + 1, 512)],
                            start=True,
                            stop=True,
                        )
                        if it in filler_iters and fq:
                            fq.pop(0)()
                        it += 1
                        if mt > 1:
                            nc.tensor.matmul(
                                ot_a,
                                lhsT=v_sb[:, ts(mt - 2, 128)],
                                rhs=sts[mt - 2][:, 0:512],
                                start=(mt - 2 == 0),
                                stop=False,
                            )
                            nc.tensor.matmul(
                                ot_b,
                                lhsT=v_sb[:, ts(mt - 2, 128)],
                                rhs=sts[mt - 2][:, 512:1024],
                                start=(mt - 2 == 0),
                                stop=False,
                            )
                        st = st_pool.tile([128, 1024], bf16, tag="st", name=f"st{h}{half}")
                        nc.scalar.activation(st[:], sp[:], AFT.Exp)
                        if mt == 0:
                            nc.vector.tensor_copy(acc[:], st[:])
                        else:
                            nc.vector.tensor_add(acc[:], st[:], acc[:])
                        sts[mt] = st
                    for k in (NT - 2, NT - 1):
                        nc.tensor.matmul(
                            ot_a,
                            lhsT=v_sb[:, ts(k, 128)],
                            rhs=sts[k][:, 0:512],
                            start=False,
                            stop=(k == NT - 1),
                        )
                        nc.tensor.matmul(
                            ot_b,
                            lhsT=v_sb[:, ts(k, 128)],
                            rhs=sts[k][:, 512:1024],
                            start=False,
                            stop=(k == NT - 1),
                        )
                    # normalize + stage for gather; sums land in a spair tile
                    # (keeps misc free for filler groups)
                    sums_ps = sp_pool.tile(
                        [128, 1024], f32, tag="spair", name=f"sums{h}{half}"
                    )
                    for j, otp in ((0, ot_a), (1, ot_b)):
                        ncx = 2 * half + j
                        nc.tensor.matmul(
                            sums_ps[:, ts(j, 512)],
                            lhsT=ones_sb[:],
                            rhs=acc[:, ts(j, 512)],
                            start=True,
                            stop=True,
                        )
                        recip = ev_pool.tile([128, 512], f32, tag="recip")
                        nc.vector.reciprocal_approx_fast(recip[:], sums_ps[:, ts(j, 512)])
                        otn = otn_pool.tile([128, 512], bf16, tag="otn")
                        nc.vector.tensor_mul(otn[:], otp, recip[:])
                        nchunks = CHUNKS[h]
                        ncol = N // nchunks  # chunk width in n-columns
                        c = (ncx * 512) // ncol
                        coff = ncx * 512 - c * ncol
                        nc.sync.dma_start(
                            ag_ins[h][c][:, coff : coff + 512], otn[:]
                        )
                        if (ncx + 1) * 512 == (c + 1) * ncol:
                            # this chunk's staging is complete: gather it and
                            # prefetch the result into this head's slice of
                            # its SBUF slab tile
                            nc.gpsimd.collective_compute(
                                "AllGather",
                                mybir.AluOpType.bypass,
                                replica_groups=[[0, 1, 2, 3], [4, 5, 6, 7]],
                                ins=[ag_ins[h][c][:].opt()],
                                outs=[ag_outs[h][c][:].opt()],
                            )
                            ag_v = ag_outs[h][c].rearrange(
                                "(r p) n -> p r n", p=128
                            )
                            nc.sync.dma_start(
                                ag_tiles[h][:, :, c * ncol : (c + 1) * ncol], ag_v[:]
                            )
                while fq:
                    fq.pop(0)()

            def start_head(h):
                ag_tiles[h] = ag_pool.tile(
                    [128, HKV, N], bf16, tag="agsb", name=f"agsb{h}"
                )

            start_head(0)
            attn_head(0, qproj_fillers(1), range(32))
            start_head(1)
            attn_head(1, qproj_fillers(2), range(32))
            start_head(2)
            attn_head(
                2,
                slab_fillers(0) + slab_fillers(1, (0, 1)) + slab_fillers(1, (2, 3)),
                range(32),
            )
            def attn_head_narrow(h, fillers, filler_iters):
                """Last-head attention as 4 narrow ncx-passes: each pass emits
                its otn + quarter-gather at 25/50/75/100% of the window so the
                final quarter's ring latency is the only unoverlapped tail."""
                it = 0
                fq = list(fillers)
                for ncx in range(NC512):
                    ot_p = ot_pool.tile(
                        [128, 512], f32, tag="otp", name=f"otn{h}{ncx}"
                    )
                    acc = acc_pool.tile(
                        [128, 1024], bf16, tag="acc", name=f"accn{h}{ncx}"
                    )[:, 0:512]
                    sts = [None] * NT
                    for mt in range(NT):
                        sp = sp_pool.tile(
                            [128, 1024], f32, tag="spair", name=f"spn{h}{ncx}"
                        )[:, 0:512]
                        nc.tensor.matmul(
                            sp,
                            lhsT=kT_sb[:, ts(mt, 128)],
                            rhs=qT_sb[:, h, ts(ncx, 512)],
                            start=True,
                            stop=True,
                        )
                        if it in filler_iters and fq:
                            fq.pop(0)()
                        it += 1
                        if mt > 1:
                            nc.tensor.matmul(
                                ot_p,
                                lhsT=v_sb[:, ts(mt - 2, 128)],
                                rhs=sts[mt - 2],
                                start=(mt - 2 == 0),
                                stop=False,
                            )
                        st = st_pool.tile(
                            [128, 1024], bf16, tag="st", name=f"stn{h}{ncx}"
                        )[:, 0:512]
                        nc.scalar.activation(st[:], sp, AFT.Exp)
                        if mt == 0:
                            nc.vector.tensor_copy(acc[:], st[:])
                        else:
                            nc.vector.tensor_add(acc[:], st[:], acc[:])
                        sts[mt] = st
                    for k in (NT - 2, NT - 1):
                        nc.tensor.matmul(
                            ot_p,
                            lhsT=v_sb[:, ts(k, 128)],
                            rhs=sts[k],
                            start=False,
                            stop=(k == NT - 1),
                        )
                    sums_ps = sp_pool.tile(
                        [128, 1024], f32, tag="spair", name=f"sumsn{h}{ncx}"
                    )[:, 0:512]
                    nc.tensor.matmul(
                        sums_ps, lhsT=ones_sb[:], rhs=acc[:], start=True, stop=True
                    )
                    recip = ev_pool.tile([128, 512], f32, tag="recip")
                    nc.vector.reciprocal_approx_fast(recip[:], sums_ps)
                    otn = otn_pool.tile([128, 512], bf16, tag="otn")
                    nc.vector.tensor_mul(otn[:], ot_p, recip[:])
                    nc.sync.dma_start(ag_ins[h][ncx][:, :], otn[:])
                    nc.gpsimd.collective_compute(
                        "AllGather",
                        mybir.AluOpType.bypass,
                        replica_groups=[[0, 1, 2, 3], [4, 5, 6, 7]],
                        ins=[ag_ins[h][ncx][:].opt()],
                        outs=[ag_outs[h][ncx][:].opt()],
                    )
                    ag_v = ag_outs[h][ncx].rearrange("(r p) n -> p r n", p=128)
                    nc.sync.dma_start(
                        ag_tiles[h][:, :, ncx * 512 : (ncx + 1) * 512], ag_v[:]
                    )
                while fq:
                    fq.pop(0)()

            start_head(3)
            attn_head_narrow(
                3,
                slab_fillers(2, (0, 1))
                + slab_fillers(2, (2, 3))
                + slab_fillers(3, (0,))
                + slab_fillers(3, (1,))
                + slab_fillers(3, (2,)),
                list(range(16, 32))
                + list(range(36, 40))
                + list(range(48, 52))
                + list(range(60, 64)),
            )
            for f in slab_fillers(3, (3,)):
                f()

    nc.compile()
    _NC_CACHE["nc"] = nc
    return nc


def kernel(x, Wq, Wk, Wv, Wo):
    _install_axon_ntff_hook()
    import ml_dtypes

    import concourse.bass_utils as bass_utils

    bass_utils.upload_artifacts = lambda tmpdir: str(tmpdir)
    from concourse.bass_utils import run_bass_kernel_spmd

    x = np.asarray(x, dtype=np.float32)
    Wq = np.asarray(Wq, dtype=np.float32)
    Wk = np.asarray(Wk, dtype=np.float32)
    Wv = np.asarray(Wv, dtype=np.float32)
    Wo = np.asarray(Wo, dtype=np.float32)

    bf = ml_dtypes.bfloat16
    scale = np.float32(HD**-0.5)
    wq_f = (_fold_rope(Wq, H) * scale).astype(bf)  # [D, 2048]
    wk_f = _fold_rope(Wk, HKV).astype(bf)  # [D, 512]
    wv_f = Wv.astype(bf)  # [D, 512]
    wo_f = Wo.astype(bf)  # [2048, D]
    cos_t = _cos_table()  # [128, N] fp32

    xt = [np.ascontiguousarray(x[b].T).astype(bf) for b in range(B)]

    in_maps = []
    for c in range(N_CORES):
        b, g = divmod(c, HKV)
        in_maps.append(
            {
                "xt": xt[b],
                "wq": np.ascontiguousarray(wq_f[:, g * JL : (g + 1) * JL]),
                "wk": np.ascontiguousarray(wk_f[:, g * HD : (g + 1) * HD]),
                "wv": np.ascontiguousarray(wv_f[:, g * HD : (g + 1) * HD]),
                "wo": np.ascontiguousarray(wo_f[:, g * DSLICE : (g + 1) * DSLICE]),
                "cost": cos_t,
            }
        )

    nc = _build_nc()
    res = run_bass_kernel_spmd(nc, in_maps, list(range(N_CORES)))

    out = np.empty((B, N, D), dtype=np.float32)
    for c in range(N_CORES):
        b, g = divmod(c, HKV)
        out[b, :, g * DSLICE : (g + 1) * DSLICE] = res.results[c]["out"].T
    return out


# revision 22
# speedup vs baseline: 1.0132x; 1.0132x over previous
"""GroupedQueryAttention (B=2, N=2048, D=2048, H=16, HKV=4, HD=128) on 8 trn2 cores.

Sharding: core c handles (batch b = c//4, kv-head g = c%4): 4 q-heads + 1 kv head.
RoPE (with the reference's sin==cos quirk) is folded into Wq/Wk host-side.
The softmax scale is folded into Wq. All matmuls run in bf16 with fp32 PSUM
accumulation, feed 512 (PSUM bank limit), kd-outer loops for stationary reuse.

Attention is restructured around engine balance (PE 2.4GHz, Scalar 1.2GHz):
per (head, n-half of 1024): scores for an ncx-pair land in one 2-bank PSUM
tile [128,1024]; ONE wide exp on ScalarE covers both (amortizes per-op
overhead); softmax denominators come from DVE bf16 accumulation of the exp
tiles (replacing 256 all-ones PE matmuls with 16) + one ones-matmul per
n-chunk; PV matmuls are software-pipelined one mt behind scores so the PE
never waits on the same-iteration exp. Per-head chunked AllGathers (bf16)
overlap with later heads' attention; gathered slabs prefetch into 4 SBUF
buffers; the output projection accumulates into a transposed [d, n] fp32
SBUF accumulator (aliased over x's SBUF slot), and the host transposes each
core's [512, 2048] slice back.
Host gathers: out[b][:, g*512:(g+1)*512] = core (b,g) output transposed.
"""

import sys
import types

import numpy as np

B, N, D = 2, 2048, 2048
H, HKV, HD = 16, 4, 128
G = H // HKV  # q heads per kv head = 4
N_CORES = 8
ROPE_BASE = 10000.0
DSLICE = D // G  # 512 output columns per core
JL = G * HD  # 512 local attention-output rows per core


def _install_axon_ntff_hook():
    """This container's antenv lacks axon_hooks; inject it so trace=True works."""
    if "antenv.axon_hooks" in sys.modules:
        return
    try:
        from trn_agent_boot.trn_boot import _ntff_profile_via_ctypes

        hook = _ntff_profile_via_ctypes("/opt/axon/libaxon_pjrt.so")
    except Exception:
        hook = None
    mod = types.ModuleType("antenv.axon_hooks")
    mod.get_axon_ntff_profile_hook = lambda: hook
    mod.set_axon_ntff_profile_hook = lambda h: None
    sys.modules["antenv.axon_hooks"] = mod


def _fold_rope(w: np.ndarray, n_heads: int) -> np.ndarray:
    """Return W' with the (sin==cos) RoPE mixing folded in: x@W' = M(x@W) per head."""
    wf = w.reshape(D, n_heads, HD)
    lo, hi = wf[..., : HD // 2], wf[..., HD // 2 :]
    return np.concatenate([lo - hi, hi + lo], axis=-1).reshape(D, n_heads * HD)


def _cos_table() -> np.ndarray:
    inv_freq = 1.0 / (ROPE_BASE ** (np.arange(0, HD, 2, dtype=np.float64) / HD))
    freqs = np.arange(N, dtype=np.float64)[:, None] * inv_freq[None, :]  # [N, 64]
    emb = np.concatenate([freqs, freqs], axis=-1)  # [N, 128]
    return np.cos(emb).T.astype(np.float32).copy()  # [128, N]


_NC_CACHE: dict = {}


def _build_nc():
    if "nc" in _NC_CACHE:
        return _NC_CACHE["nc"]

    import concourse.bacc as bacc
    import concourse.mybir as mybir
    import concourse.tile as tile
    from concourse.bass import ts
    from concourse.masks import make_identity

    f32 = mybir.dt.float32
    bf16 = mybir.dt.bfloat16
    AFT = mybir.ActivationFunctionType
    KD = D // 128  # 16 contraction chunks
    NT = N // 128  # 16 m tiles of 128
    NC512 = N // 512  # 4 chunks of 512
    DC = DSLICE // 128  # 4 output-column chunks of 128

    nc = bacc.Bacc(target_bir_lowering=False, debug=False, num_devices=N_CORES)

    xt = nc.dram_tensor("xt", [D, N], bf16, kind="ExternalInput")  # x[b].T
    wq = nc.dram_tensor("wq", [D, JL], bf16, kind="ExternalInput")  # folded+scaled
    wk = nc.dram_tensor("wk", [D, HD], bf16, kind="ExternalInput")  # folded
    wv = nc.dram_tensor("wv", [D, HD], bf16, kind="ExternalInput")
    wo = nc.dram_tensor("wo", [H * HD, DSLICE], bf16, kind="ExternalInput")
    cost = nc.dram_tensor("cost", [HD, N], f32, kind="ExternalInput")
    # transposed output: outT[d, n]; host transposes back
    out = nc.dram_tensor("out", [DSLICE, N], f32, kind="ExternalOutput")

    xt_v = xt.rearrange("(ko p) n -> p ko n", p=128)
    wq_v = wq.rearrange("(ko p) j -> p ko j", p=128)
    wk_v = wk.rearrange("(ko p) j -> p ko j", p=128)
    wv_v = wv.rearrange("(ko p) j -> p ko j", p=128)
    wo_v = wo.rearrange("(ko p) d -> p ko d", p=128)

    with tile.TileContext(nc) as tc:
        with (
            tc.tile_pool(name="big", bufs=1) as big_pool,
            tc.tile_pool(name="ag", bufs=3) as ag_pool,
            tc.tile_pool(name="otn", bufs=4) as otn_pool,
            tc.tile_pool(name="wpool", bufs=1) as w_pool,
            tc.tile_pool(name="work", bufs=1) as work_pool,
            tc.tile_pool(name="st", bufs=4) as st_pool,
            tc.tile_pool(name="accp", bufs=2) as acc_pool,
            tc.tile_pool(name="ev", bufs=2) as ev_pool,
            tc.tile_pool(name="spair", bufs=2, space="PSUM") as sp_pool,
            tc.tile_pool(name="otp", bufs=2, space="PSUM") as ot_pool,
            tc.tile_pool(name="misc", bufs=2, space="PSUM") as misc_pool,
            tc.tile_pool(name="dram", bufs=1, space="DRAM") as dram_pool,
        ):
            # ---- persistent SBUF tensors ----
            x_sb = big_pool.tile([128, KD, N], bf16, tag="big")
            wq_sb = w_pool.tile([128, KD, JL], bf16, tag="wq")
            wk_sb = w_pool.tile([128, KD, HD], bf16, tag="wk")
            wv_sb = w_pool.tile([128, KD, HD], bf16, tag="wv")
            wo_sb = w_pool.tile([128, KD, DSLICE], bf16, tag="wo")
            cos_sb = w_pool.tile([128, N], f32, tag="cos")
            qT_sb = work_pool.tile([128, G, N], bf16, tag="qT")
            kT_sb = work_pool.tile([128, N], bf16, tag="kT")
            vT_sb = ag_pool.tile([128, HKV, N], bf16, tag="agsb", name="vT_sb")[:, 0, :]
            v_sb = work_pool.tile([128, N], bf16, tag="v")  # [m-part, mt*128+hd]
            ones_sb = work_pool.tile([128, 128], bf16, tag="ones")
            ident_sb = work_pool.tile([128, 128], bf16, tag="ident")

            nc.gpsimd.memset(ones_sb[:], 1.0)
            make_identity(nc, ident_sb[:])

            # ---- input DMAs (weights needed first come first) ----
            nc.sync.dma_start(wk_sb[:], wk_v[:])
            for kd in range(4):
                nc.sync.dma_start(x_sb[:, kd, :], xt_v[:, kd, :])
            nc.sync.dma_start(cos_sb[:], cost[:, :])
            nc.sync.dma_start(wv_sb[:], wv_v[:])
            for kd in range(4, KD):
                nc.sync.dma_start(x_sb[:, kd, :], xt_v[:, kd, :])
            nc.sync.dma_start(wq_sb[:], wq_v[:])
            nc.sync.dma_start(wo_sb[:], wo_v[:])

            def proj_pair(lhs_chunks, dst_wide_op, name):
                """kd-outer projection into 2 spair tiles (4 bank-groups);
                dst_wide_op(j, psum_tile_1024) consumes each half."""
                psA = sp_pool.tile([128, 1024], f32, tag="spair", name=f"{name}A")
                psB = sp_pool.tile([128, 1024], f32, tag="spair", name=f"{name}B")
                quads = [(psA, 0), (psA, 512), (psB, 0), (psB, 512)]
                for kd in range(KD):
                    for j, (pst, off) in enumerate(quads):
                        nc.tensor.matmul(
                            pst[:, off : off + 512],
                            lhsT=lhs_chunks(kd),
                            rhs=x_sb[:, kd, ts(j, 512)],
                            start=(kd == 0),
                            stop=(kd == KD - 1),
                        )
                dst_wide_op(0, psA)
                dst_wide_op(1, psB)

            # k: kT = cos * (Wk.T @ x)
            proj_pair(
                lambda kd: wk_sb[:, kd, :],
                lambda j, ps: nc.vector.tensor_mul(
                    kT_sb[:, ts(j, 1024)], ps, cos_sb[:, ts(j, 1024)]
                ),
                "kproj",
            )

            # v: vT then PE-transpose into natural [m, hd] layout
            proj_pair(
                lambda kd: wv_sb[:, kd, :],
                lambda j, ps: nc.vector.tensor_copy(vT_sb[:, ts(j, 1024)], ps),
                "vproj",
            )
            for q4 in range(NT // 4):
                trf = misc_pool.tile([128, 512], f32, tag="misc", name="tr")
                trb = trf[:, 0:256].bitcast(bf16)
                for j in range(4):
                    mt = q4 * 4 + j
                    nc.tensor.transpose(
                        trb[:, ts(j, 128)], vT_sb[:, ts(mt, 128)], ident_sb[:]
                    )
                nc.vector.tensor_copy(v_sb[:, ts(q4, 512)], trb[:])

            # q heads 0 and 3 sequential; heads 1 and 2 are interleaved into
            # attention as filler matmuls (see qproj_fillers)
            for h in (0, 3):
                proj_pair(
                    lambda kd, h=h: wq_sb[:, kd, ts(h, 128)],
                    lambda j, ps, h=h: nc.vector.tensor_mul(
                        qT_sb[:, h, ts(j, 1024)], ps, cos_sb[:, ts(j, 1024)]
                    ),
                    f"qproj{h}",
                )

            # outT fp32 accumulator for the output projection: aliases x_sb's
            # SBUF slot (same tag, bufs=1); x is dead after qproj.
            outT_acc = big_pool.tile([128, DC, N], f32, tag="big")

            # ---- collective staging ----
            # staging chunks: head h is gathered in CHUNKS[h] pieces so ring
            # latency overlaps production (smaller chunks for later heads)
            CHUNKS = [1, 2, 2, 4]
            ag_ins = []   # [h][c] -> dram [HD, chunk_n]
            ag_outs = []  # [h][c] -> dram [HKV*HD, chunk_n]
            for h in range(G):
                cn = N // CHUNKS[h]
                ag_ins.append(
                    [
                        dram_pool.tile(
                            [HD, cn], bf16, tag=f"agi{h}_{c}", name=f"agi{h}_{c}"
                        )
                        for c in range(CHUNKS[h])
                    ]
                )
                ag_outs.append(
                    [
                        dram_pool.tile(
                            [HKV * HD, cn],
                            bf16,
                            tag=f"ago{h}_{c}",
                            name=f"ago{h}_{c}",
                        )
                        for c in range(CHUNKS[h])
                    ]
                )

            # leading tiny collective: absorbs cross-core rendezvous skew while
            # the PE is busy with projections, so the first real gather is cheap
            bar_in = dram_pool.tile([1, 128], bf16, tag="bar_in", name="bar_in")
            bar_out = dram_pool.tile([4, 128], bf16, tag="bar_out", name="bar_out")
            nc.gpsimd.collective_compute(
                "AllGather",
                mybir.AluOpType.bypass,
                replica_groups=[[0, 1, 2, 3], [4, 5, 6, 7]],
                ins=[bar_in[:].opt()],
                outs=[bar_out[:].opt()],
            )

            ag_tiles = {}

            def qproj_fillers(h):
                """32 closures: qproj for head h via misc-pool kd-groups.
                4 ncx-groups of 16 matmuls, 2 per closure (8 closures/group);
                groups close before half boundaries so misc rotation is safe."""
                fns = []
                for ncx in range(NC512):
                    qps = misc_pool.tile(
                        [128, 512], f32, tag="misc", name=f"qf{h}{ncx}"
                    )
                    for k2 in range(8):

                        def emit(h=h, ncx=ncx, k2=k2, qps=qps):
                            for kd in (2 * k2, 2 * k2 + 1):
                                nc.tensor.matmul(
                                    qps,
                                    lhsT=wq_sb[:, kd, ts(h, 128)],
                                    rhs=x_sb[:, kd, ts(ncx, 512)],
                                    start=(kd == 0),
                                    stop=(kd == KD - 1),
                                )
                            if k2 == 7:
                                nc.vector.tensor_mul(
                                    qT_sb[:, h, ts(ncx, 512)],
                                    qps,
                                    cos_sb[:, ts(ncx, 512)],
                                )

                        fns.append(emit)
                return fns

            def slab_fillers(h, nn_range=None):
                """Closures: output-projection slab for head h; each closure is
                one complete 4-matmul misc-pool group + DVE accumulate. For
                h==3 the gathered slab lives in two n-chunk tiles. Loops nn
                outer so chunk-0 groups come first (gather-chunk gating)."""
                fns = []
                for nn in nn_range if nn_range is not None else range(NC512):
                    for dc in range(DC):

                        def emit(h=h, dc=dc, nn=nn):
                            ps = misc_pool.tile(
                                [128, 512], f32, tag="misc", name=f"sd{h}"
                            )
                            for r in range(HKV):
                                jc = r * G + h
                                rhs = ag_tiles[h][:, r, ts(nn, 512)]
                                nc.tensor.matmul(
                                    ps,
                                    lhsT=wo_sb[:, jc, ts(dc, 128)],
                                    rhs=rhs,
                                    start=(r == 0),
                                    stop=(r == HKV - 1),
                                )
                            if h == 0:
                                nc.vector.tensor_copy(outT_acc[:, dc, ts(nn, 512)], ps)
                            else:
                                nc.vector.tensor_add(
                                    outT_acc[:, dc, ts(nn, 512)],
                                    ps,
                                    outT_acc[:, dc, ts(nn, 512)],
                                )
                            if h == G - 1 and nn == 1:
                                nc.sync.dma_start(
                                    out[ts(dc, 128), 0:1024],
                                    outT_acc[:, dc, 0:1024],
                                )
                            if h == G - 1 and nn == NC512 - 1:
                                nc.sync.dma_start(
                                    out[ts(dc, 128), 1024:2048],
                                    outT_acc[:, dc, 1024:2048],
                                )

                        fns.append(emit)
                return fns

            def attn_head(h, fillers, filler_iters):
                """Attention for head h; drains one filler closure per
                iteration index listed in filler_iters (0..31)."""
                it = 0
                fq = list(fillers)
                for half in range(2):
                    ot_a = ot_pool.tile([128, 512], f32, tag="otp", name=f"ota{h}{half}")
                    ot_b = ot_pool.tile([128, 512], f32, tag="otp", name=f"otb{h}{half}")
                    acc = acc_pool.tile([128, 1024], bf16, tag="acc", name=f"acc{h}{half}")
                    sts = [None] * NT
                    for mt in range(NT):
                        sp = sp_pool.tile(
                            [128, 1024], f32, tag="spair", name=f"sp{h}{half}"
                        )
                        nc.tensor.matmul(
                            sp[:, 0:512],
                            lhsT=kT_sb[:, ts(mt, 128)],
                            rhs=qT_sb[:, h, ts(2 * half, 512)],
                            start=True,
                            stop=True,
                        )
                        nc.tensor.matmul(
                            sp[:, 512:1024],
                            lhsT=kT_sb[:, ts(mt, 128)],
                            rhs=qT_sb[:, h, ts(2 * half + 1, 512)],
                            start=True,
                            stop=True,
                        )
                        if it in filler_iters and fq:
                            fq.pop(0)()
                        it += 1
                        if mt > 1:
                            nc.tensor.matmul(
                                ot_a,
                                lhsT=v_sb[:, ts(mt - 2, 128)],
                                rhs=sts[mt - 2][:, 0:512],
                                start=(mt - 2 == 0),
                                stop=False,
                            )
                            nc.tensor.matmul(
                                ot_b,
                                lhsT=v_sb[:, ts(mt - 2, 128)],
                                rhs=sts[mt - 2][:, 512:1024],
                                start=(mt - 2 == 0),
                                stop=False,
                            )
                        st = st_pool.tile([128, 1024], bf16, tag="st", name=f"st{h}{half}")
                        nc.scalar.activation(st[:], sp[:], AFT.Exp)
                        if mt == 0:
                            nc.vector.tensor_copy(acc[:], st[:])
                        else:
                            nc.vector.tensor_add(acc[:], st[:], acc[:])
                        sts[mt] = st
                    for k in (NT - 2, NT - 1):
                        nc.tensor.matmul(
                            ot_a,
                            lhsT=v_sb[:, ts(k, 128)],
                            rhs=sts[k][:, 0:512],
                            start=False,
                            stop=(k == NT - 1),
                        )
                        nc.tensor.matmul(
                            ot_b,
                            lhsT=v_sb[:, ts(k, 128)],
                            rhs=sts[k][:, 512:1024],
                            start=False,
                            stop=(k == NT - 1),
                        )
                    # normalize + stage for gather; sums land in a spair tile
                    # (keeps misc free for filler groups)
                    sums_ps = sp_pool.tile(
                        [128, 1024], f32, tag="spair", name=f"sums{h}{half}"
                    )
                    for j, otp in ((0, ot_a), (1, ot_b)):
                        ncx = 2 * half + j
                        nc.tensor.matmul(
                            sums_ps[:, ts(j, 512)],
                            lhsT=ones_sb[:],
                            rhs=acc[:, ts(j, 512)],
                            start=True,
                            stop=True,
                        )
                        recip = ev_pool.tile([128, 512], f32, tag="recip")
                        nc.vector.reciprocal_approx_fast(recip[:], sums_ps[:, ts(j, 512)])
                        otn = otn_pool.tile([128, 512], bf16, tag="otn")
                        nc.vector.tensor_mul(otn[:], otp, recip[:])
                        nchunks = CHUNKS[h]
                        ncol = N // nchunks  # chunk width in n-columns
                        c = (ncx * 512) // ncol
                        coff = ncx * 512 - c * ncol
                        nc.sync.dma_start(
                            ag_ins[h][c][:, coff : coff + 512], otn[:]
                        )
                        if (ncx + 1) * 512 == (c + 1) * ncol:
                            # this chunk's staging is complete: gather it and
                            # prefetch the result into this head's slice of
                            # its SBUF slab tile
                            nc.gpsimd.collective_compute(
                                "AllGather",
                                mybir.AluOpType.bypass,
                                replica_groups=[[0, 1, 2, 3], [4, 5, 6, 7]],
                                ins=[ag_ins[h][c][:].opt()],
                                outs=[ag_outs[h][c][:].opt()],
                            )
                            ag_v = ag_outs[h][c].rearrange(
                                "(r p) n -> p r n", p=128
                            )
                            nc.sync.dma_start(
                                ag_tiles[h][:, :, c * ncol : (c + 1) * ncol], ag_v[:]
                            )
                while fq:
                    fq.pop(0)()

            def start_head(h):
                ag_tiles[h] = ag_pool.tile(
                    [128, HKV, N], bf16, tag="agsb", name=f"agsb{h}"
                )

            start_head(0)
            attn_head(0, qproj_fillers(1), range(32))
            start_head(1)
            attn_head(1, qproj_fillers(2), range(32))
            start_head(2)
            attn_head(
                2,
                slab_fillers(0) + slab_fillers(1, (0, 1)) + slab_fillers(1, (2, 3)),
                range(32),
            )
            def attn_head_narrow(h, fillers, filler_iters):
                """Last-head attention as 4 narrow ncx-passes: each pass emits
                its otn + quarter-gather at 25/50/75/100% of the window so the
                final quarter's ring latency is the only unoverlapped tail."""
                it = 0
                fq = list(fillers)
                for ncx in range(NC512):
                    ot_p = ot_pool.tile(
                        [128, 512], f32, tag="otp", name=f"otn{h}{ncx}"
                    )
                    acc = acc_pool.tile(
                        [128, 1024], bf16, tag="acc", name=f"accn{h}{ncx}"
                    )[:, 0:512]
                    sts = [None] * NT
                    for mt in range(NT):
                        sp = sp_pool.tile(
                            [128, 1024], f32, tag="spair", name=f"spn{h}{ncx}"
                        )[:, 0:512]
                        nc.tensor.matmul(
                            sp,
                            lhsT=kT_sb[:, ts(mt, 128)],
                            rhs=qT_sb[:, h, ts(ncx, 512)],
                            start=True,
                            stop=True,
                        )
                        if it in filler_iters and fq:
                            fq.pop(0)()
                        it += 1
                        if mt > 1:
                            nc.tensor.matmul(
                                ot_p,
                                lhsT=v_sb[:, ts(mt - 2, 128)],
                                rhs=sts[mt - 2],
                                start=(mt - 2 == 0),
                                stop=False,
                            )
                        st = st_pool.tile(
                            [128, 1024], bf16, tag="st", name=f"stn{h}{ncx}"
                        )[:, 0:512]
                        nc.scalar.activation(st[:], sp, AFT.Exp)
                        if mt == 0:
                            nc.vector.tensor_copy(acc[:], st[:])
                        else:
                            nc.vector.tensor_add(acc[:], st[:], acc[:])
                        sts[mt] = st
                    for k in (NT - 2, NT - 1):
                        nc.tensor.matmul(
                            ot_p,
                            lhsT=v_sb[:, ts(k, 128)],
                            rhs=sts[k],
                            start=False,
                            stop=(k == NT - 1),
                        )
                    sums_ps = sp_pool.tile(
                        [128, 1024], f32, tag="spair", name=f"sumsn{h}{ncx}"
                    )[:, 0:512]
                    nc.tensor.matmul(
                        sums_ps, lhsT=ones_sb[:], rhs=acc[:], start=True, stop=True
                    )
                    recip = ev_pool.tile([128, 512], f32, tag="recip")
                    nc.vector.reciprocal_approx_fast(recip[:], sums_ps)
                    otn = otn_pool.tile([128, 512], bf16, tag="otn")
                    nc.vector.tensor_mul(otn[:], ot_p, recip[:])
                    nc.sync.dma_start(ag_ins[h][ncx][:, :], otn[:])
                    nc.gpsimd.collective_compute(
                        "AllGather",
                        mybir.AluOpType.bypass,
                        replica_groups=[[0, 1, 2, 3], [4, 5, 6, 7]],
                        ins=[ag_ins[h][ncx][:].opt()],
                        outs=[ag_outs[h][ncx][:].opt()],
                    )
                    ag_v = ag_outs[h][ncx].rearrange("(r p) n -> p r n", p=128)
                    nc.sync.dma_start(
                        ag_tiles[h][:, :, ncx * 512 : (ncx + 1) * 512], ag_v[:]
                    )
                while fq:
                    fq.pop(0)()

            start_head(3)
            attn_head_narrow(
                3,
                slab_fillers(2, (0, 1))
                + slab_fillers(2, (2, 3))
                + slab_fillers(3, (0,))
                + slab_fillers(3, (1,)),
                list(range(16, 32))
                + list(range(36, 40))
                + list(range(48, 52)),
            )
            for f in slab_fillers(3, (2,)) + slab_fillers(3, (3,)):
                f()

    nc.compile()
    _NC_CACHE["nc"] = nc
    return nc


def kernel(x, Wq, Wk, Wv, Wo):
    _install_axon_ntff_hook()
    import ml_dtypes

    import concourse.bass_utils as bass_utils

    bass_utils.upload_artifacts = lambda tmpdir: str(tmpdir)
    from concourse.bass_utils import run_bass_kernel_spmd

    x = np.asarray(x, dtype=np.float32)
    Wq = np.asarray(Wq, dtype=np.float32)
    Wk = np.asarray(Wk, dtype=np.float32)
    Wv = np.asarray(Wv, dtype=np.float32)
    Wo = np.asarray(Wo, dtype=np.float32)

    bf = ml_dtypes.bfloat16
    scale = np.float32(HD**-0.5)
    wq_f = (_fold_rope(Wq, H) * scale).astype(bf)  # [D, 2048]
    wk_f = _fold_rope(Wk, HKV).astype(bf)  # [D, 512]
    wv_f = Wv.astype(bf)  # [D, 512]
    wo_f = Wo.astype(bf)  # [2048, D]
    cos_t = _cos_table()  # [128, N] fp32

    xt = [np.ascontiguousarray(x[b].T).astype(bf) for b in range(B)]

    in_maps = []
    for c in range(N_CORES):
        b, g = divmod(c, HKV)
        in_maps.append(
            {
                "xt": xt[b],
                "wq": np.ascontiguousarray(wq_f[:, g * JL : (g + 1) * JL]),
                "wk": np.ascontiguousarray(wk_f[:, g * HD : (g + 1) * HD]),
                "wv": np.ascontiguousarray(wv_f[:, g * HD : (g + 1) * HD]),
                "wo": np.ascontiguousarray(wo_f[:, g * DSLICE : (g + 1) * DSLICE]),
                "cost": cos_t,
            }
        )

    nc = _build_nc()
    res = run_bass_kernel_spmd(nc, in_maps, list(range(N_CORES)))

    out = np.empty((B, N, D), dtype=np.float32)
    for c in range(N_CORES):
        b, g = divmod(c, HKV)
        out[b, :, g * DSLICE : (g + 1) * DSLICE] = res.results[c]["out"].T
    return out
